# revision 15
# baseline (speedup 1.0000x reference)
"""Distributed GAT (2-layer, PyG GATConv semantics) as a Bass/Tile SPMD kernel
for 8 Trainium2 NeuronCores — v2.

Changes vs v1 (kernel.py):
  - The per-edge alpha_dst gather (adt, 256B rows, ~half of all gathered rows)
    is eliminated. Edge-phase DMA is per-row-cost-bound (~10ns/row regardless
    of row size), so halving row count ~halves edge DMA time. alpha_dst is now
    computed on-chip: a dst one-hot selector transposed to dst-major (StT) is
    matmul'd against the group's dense 128x8 adst tile (kept in SBUF since the
    table build). StT is built cheaply: is_equal against a host-precomputed
    block-swizzled dst array, then one DVE StreamTranspose (32x32 blocks)
    turns the swizzled one-hot into the true transpose. No PE transposes, no
    PSUM round-trips.
  - Per-group subtile counts UA[g]/UB[g] (max over cores) instead of global
    maxima: ~10% fewer gathered rows. Index/dst arrays are stored as flat
    ragged per-group slabs so every DMA stays a single contiguous block.
"""
import math
import numpy as np

import concourse.bass as bass
import concourse.bacc as bacc
import concourse.tile as tile
from concourse import mybir
from concourse.masks import make_identity

F32 = mybir.dt.float32
F32R = mybir.dt.float32r
BF16 = mybir.dt.bfloat16
F8 = mybir.dt.float8e4
I16 = mybir.dt.int16

P = 128
SPLIT_AT = 32768          # int16 index limit for dma_gather


class Cfg:
    def __init__(self, N, DIN, H, C, OUT, n_cores):
        self.N, self.DIN, self.H, self.C, self.OUT = N, DIN, H, C, OUT
        self.HID = H * C
        self.GC = self.HID + H              # useful gathered cols: h | asrc
        self.RT = 320                       # table-build psum row (f32)
        self.RTB = 384                      # bf16 table row (768B % 256 == 0)
        assert self.HID + 2 * H <= self.RT
        self.n_cores = n_cores
        assert N % n_cores == 0
        self.npc = N // n_cores
        assert self.npc <= SPLIT_AT, "local dst must fit int16"
        self.NB = math.ceil(self.npc / P)
        self.npc_pad = self.NB * P
        self.split = N > SPLIT_AT
        # filled by preprocess: per-group subtile counts (uniform over cores)
        self.UAg = None     # [NB] int
        self.UBg = None     # [NB] int
        self.offA = None    # [NB] slab offsets (subtile units)
        self.offB = None
        self.offT = None


def _wrap_idx(vals):
    """int16 index list (len % 16 == 0) -> dma_gather wrapped layout
    [128, len/16]: index j at partition j%16 col j//16, replicated x8."""
    n = len(vals)
    w = vals.reshape(n // 16, 16).T.astype(np.int16)   # [16, n/16]
    return np.tile(w, (8, 1))                          # [128, n/16]


def preprocess(cfg: Cfg, edge_index: np.ndarray):
    """Per-core edge-stream arrays for the group-wise dma_gathers.

    Returns list per core of dict (flat ragged slabs, one per group):
      idxA [P * sum(UAg) * 8] i16   (src < SPLIT_AT)
      idxB [P * sum(UBg) * 8] i16   (src - SPLIT_AT)   (only if cfg.split)
      st   [P, sum(UTg) * P] fp8    one-hot scatter matrices, partition-major:
        st[p, (offT[g]+j)*P + d] = 1 iff edge slot (p, j) of group g targets
        group-local dst d; all-zero row for padding slots.
      stt  [P, sum(UTg) * P] fp8    per-subtile transposes:
        stt[d, (offT[g]+j)*P + p] = st[p, (offT[g]+j)*P + d]
    """
    N, n_cores, npc = cfg.N, cfg.n_cores, cfg.npc
    NB, npc_pad = cfg.NB, cfg.npc_pad

    src = np.concatenate([edge_index[0], np.arange(N, dtype=edge_index.dtype)])
    dst = np.concatenate([edge_index[1], np.arange(N, dtype=edge_index.dtype)])
    order = np.argsort(dst, kind="stable")
    src_s = np.asarray(src[order], dtype=np.int64)
    dst_s = np.asarray(dst[order], dtype=np.int64)
    bounds = np.searchsorted(dst_s, np.arange(n_cores + 1) * npc)

    cores = []
    cntA_all = np.zeros((n_cores, NB), np.int64)
    cntB_all = np.zeros((n_cores, NB), np.int64)
    for c in range(n_cores):
        lo, hi = bounds[c], bounds[c + 1]
        s_c = src_s[lo:hi]
        d_c = dst_s[lo:hi] - c * npc
        if npc_pad > npc:  # fake dst rows so every psum row has a real denom
            fake = np.arange(npc, npc_pad, dtype=np.int64)
            s_c = np.concatenate([s_c, np.zeros(len(fake), np.int64)])
            d_c = np.concatenate([d_c, fake])
        isB = (s_c >= SPLIT_AT) if cfg.split else np.zeros(len(s_c), bool)
        g_c = d_c // P
        # sort by (group, section, dst)
        key = (g_c * 2 + isB) * npc_pad + d_c
        o = np.argsort(key, kind="stable")
        s_c, d_c, g_c, isB = s_c[o], d_c[o], g_c[o], isB[o]
        cntA_all[c] = np.bincount(g_c[~isB], minlength=NB)
        cntB_all[c] = np.bincount(g_c[isB], minlength=NB)
        cores.append((s_c, d_c, g_c, isB))

    UAg = np.maximum(np.ceil(cntA_all.max(axis=0) / P).astype(np.int64), 1)
    if cfg.split:
        UBg = np.maximum(np.ceil(cntB_all.max(axis=0) / P).astype(np.int64), 1)
    else:
        UBg = np.zeros(NB, np.int64)
    cfg.UAg, cfg.UBg = UAg, UBg
    UTg = UAg + UBg
    cfg.offA = np.concatenate([[0], np.cumsum(UAg)[:-1]])
    cfg.offB = np.concatenate([[0], np.cumsum(UBg)[:-1]])
    cfg.offT = np.concatenate([[0], np.cumsum(UTg)[:-1]])

    # stream slot bases per group
    baseT = np.zeros(NB, np.int64)
    baseT[1:] = np.cumsum(UTg * P)[:-1]
    total_slots = int((UTg * P).sum())

    out = []
    for c, (s_c, d_c, g_c, isB) in enumerate(cores):
        cntA = cntA_all[c]
        cntB = cntB_all[c]
        startA = np.zeros(NB + 1, np.int64)
        np.cumsum(cntA, out=startA[1:])
        startB = np.zeros(NB + 1, np.int64)
        np.cumsum(cntB, out=startB[1:])
        rank = np.empty(len(d_c), np.int64)
        idxall = np.arange(len(d_c), dtype=np.int64)
        secA = ~isB
        # edges sorted by (group, section, dst): rank within own section
        rank[secA] = idxall[secA] - (startA[g_c[secA]] + startB[g_c[secA]])
        rank[isB] = idxall[isB] - (startA[g_c[isB] + 1] + startB[g_c[isB]])
        tgt = baseT[g_c] + np.where(isB, UAg[g_c] * P + rank, rank)

        srcv = np.zeros(total_slots, np.int64)          # pad -> row 0
        dstcv = np.full(total_slots, -1.0, np.float32)  # pad -> no dst
        srcv[tgt] = np.where(isB, s_c - SPLIT_AT, s_c)
        dstcv[tgt] = (d_c % P).astype(np.float32)

        idxA = np.zeros(P * int(UAg.sum()) * 8, np.int16)
        idxB = np.zeros(P * int(UBg.sum()) * 8, np.int16)
        sumUT = int(UTg.sum())
        # one-hot scatter matrices as raw fp8e4 bit patterns (1.0 = 0x38)
        st = np.zeros((P, sumUT, P), np.uint8)
        stt = np.zeros((P, sumUT, P), np.uint8)
        for g in range(NB):
            ua, ub, ut = int(UAg[g]), int(UBg[g]), int(UTg[g])
            sl = srcv[baseT[g]:baseT[g] + ut * P]
            dl = dstcv[baseT[g]:baseT[g] + ut * P]
            oa = int(cfg.offA[g]) * P * 8
            idxA[oa:oa + ua * P * 8] = _wrap_idx(sl[:ua * P]).ravel()
            if ub:
                ob = int(cfg.offB[g]) * P * 8
                idxB[ob:ob + ub * P * 8] = _wrap_idx(sl[ua * P:]).ravel()
            # dc[p, j] = group-local dst of edge slot (p, j), -1 for padding
            dc = dl.reshape(ut, P).T.astype(np.int64)     # [P, ut]
            ot = int(cfg.offT[g])
            onehot = (dc[:, :, None] == np.arange(P)[None, None, :])
            st[:, ot:ot + ut, :] = onehot * np.uint8(0x38)
            stt[:, ot:ot + ut, :] = onehot.transpose(2, 1, 0) * np.uint8(0x38)
        d = {"idxA": idxA,
             "st": np.ascontiguousarray(st.reshape(P, sumUT * P)),
             "stt": np.ascontiguousarray(stt.reshape(P, sumUT * P))}
        if cfg.split:
            d["idxB"] = idxB
        out.append(d)
    return out


def expand_att(a, HID, H, C):
    A = np.zeros((HID, H), np.float32)
    for h in range(H):
        A[h * C:(h + 1) * C, h] = a[h]
    return A


def build_program(cfg: Cfg, edge_reps=1, whole_reps=1, no_collectives=False,
                  pert=frozenset(),
                  gw_bufs=3, idx_bufs=4, sq=4, f32r=False, ch=8):
    """Emit the (core-uniform) SPMD program. Returns nc."""
    NB = cfg.NB
    UAg, UBg = cfg.UAg, cfg.UBg
    RT, RTB, GC = cfg.RT, cfg.RTB, cfg.GC
    HID, OUT, DIN, H = cfg.HID, cfg.OUT, cfg.DIN, cfg.H
    npc, N = cfg.npc, cfg.N
    DC = DIN // P
    HC = HID // P
    NA_ROWS = min(N, SPLIT_AT)
    lenA = P * int(UAg.sum()) * 8
    lenB = P * int(UBg.sum()) * 8
    sumUT = int((UAg + UBg).sum())

    nc = bacc.Bacc("TRN2", target_bir_lowering=False, debug=False,
                   num_devices=cfg.n_cores, num_swdge_queues=sq)

    t_xT = nc.dram_tensor("xT", [DIN, npc], F32, kind="ExternalInput")
    t_M1 = nc.dram_tensor("M1", [DIN, RT], F32, kind="ExternalInput")
    t_M2 = nc.dram_tensor("M2", [HID, RT], F32, kind="ExternalInput")
    t_Wc = nc.dram_tensor("Wc", [HID, OUT], F32, kind="ExternalInput")
    t_b1 = nc.dram_tensor("b1", [P, HID], F32, kind="ExternalInput")
    t_b2 = nc.dram_tensor("b2", [P, HID], F32, kind="ExternalInput")
    t_bc = nc.dram_tensor("bc", [P, OUT], F32, kind="ExternalInput")
    t_idxA = nc.dram_tensor("idxA", [lenA], I16, kind="ExternalInput")
    if cfg.split:
        t_idxB = nc.dram_tensor("idxB", [lenB], I16, kind="ExternalInput")
    t_st = nc.dram_tensor("st", [P, sumUT * P], F8, kind="ExternalInput")
    t_stt = nc.dram_tensor("stt", [P, sumUT * P], F8, kind="ExternalInput")
    t_out = nc.dram_tensor("out", [npc, OUT], F32, kind="ExternalOutput")

    rgroups = [list(range(cfg.n_cores))]

    with tile.TileContext(nc) as tc:
        with (
            tc.tile_pool(name="const", bufs=1) as cp,
            tc.tile_pool(name="xt", bufs=2) as xtp,
            tc.tile_pool(name="tbl", bufs=3) as tblp,
            tc.tile_pool(name="gw", bufs=gw_bufs) as gwp,
            tc.tile_pool(name="idx", bufs=idx_bufs) as idxp,
            tc.tile_pool(name="zz", bufs=3) as zzp,
            tc.tile_pool(name="smat", bufs=gw_bufs) as sp,
            tc.tile_pool(name="stt", bufs=gw_bufs) as stp,
            tc.tile_pool(name="xb", bufs=2) as xbp,
            tc.tile_pool(name="xtb", bufs=2) as xtbp,
            tc.tile_pool(name="hd", bufs=2) as hdp,
            tc.tile_pool(name="ps_acc", bufs=2, space="PSUM") as ps_acc,
            tc.tile_pool(name="ps_tp", bufs=2, space="PSUM") as ps_tp,
            tc.tile_pool(name="ps_ad", bufs=2, space="PSUM") as ps_ad,
            tc.tile_pool(name="ps_tb", bufs=2, space="PSUM") as ps_tb,
            tc.tile_pool(name="dram", bufs=1, space="DRAM") as dp,
        ):
            # ---- constants ----
            M1sb = cp.tile([P, DC, RT], F32)
            M2sb = cp.tile([P, HC, RT], F32)
            WcSb = cp.tile([P, HC, OUT], F32)
            b1sb = cp.tile([P, HID], F32)
            b2sb = cp.tile([P, HID], F32)
            bcsb = cp.tile([P, OUT], F32)
            ident = cp.tile([P, P], F32)
            # per-layer dense adst for own rows: [P, NB, H], col g = group g
            adst1 = cp.tile([P, NB, H], BF16)
            adst2 = cp.tile([P, NB, H], BF16)
            nc.sync.dma_start(out=M1sb[:], in_=t_M1[:, :].rearrange(
                "(a c) r -> c a r", c=P))
            nc.sync.dma_start(out=M2sb[:], in_=t_M2[:, :].rearrange(
                "(a c) r -> c a r", c=P))
            nc.sync.dma_start(out=WcSb[:], in_=t_Wc[:, :].rearrange(
                "(a c) r -> c a r", c=P))
            nc.sync.dma_start(out=b1sb[:], in_=t_b1[:, :])
            nc.sync.dma_start(out=b2sb[:], in_=t_b2[:, :])
            nc.sync.dma_start(out=bcsb[:], in_=t_bc[:, :])
            make_identity(nc, ident[:])

            # ---- internal DRAM ----
            ag1_in = dp.tile([npc, RTB], BF16)
            ag2_in = dp.tile([npc, RTB], BF16)
            table1 = dp.tile([N, RTB], BF16)
            table2 = dp.tile([N, RTB], BF16)

            # ---- phase B: layer-1 table ----
            def phase_b():
                for b in range(NB):
                    ncols = min(P, npc - b * P)
                    xt = xtp.tile([P, DC, P], F32, name="xt")
                    nc.sync.dma_start(
                        out=xt[:, :, 0:ncols],
                        in_=t_xT[:, b * P:b * P + ncols].rearrange(
                            "(a c) n -> c a n", c=P))
                    pstb = ps_tb.tile([P, RT], F32, name="pstb", tag="pstb")
                    for a in range(DC):
                        lh, rh = xt[:, a, 0:ncols], M1sb[:, a, :]
                        if f32r:
                            lh, rh = lh.bitcast(F32R), rh.bitcast(F32R)
                        nc.tensor.matmul(pstb[0:ncols, :], lh, rh,
                                         start=(a == 0), stop=(a == DC - 1))
                    tbs = tblp.tile([P, RTB], BF16, name="tbs")
                    nc.scalar.activation(tbs[0:ncols, 0:RT], pstb[0:ncols, :],
                                         mybir.ActivationFunctionType.Copy)
                    nc.sync.dma_start(out=ag1_in[b * P:b * P + ncols, 0:RT],
                                      in_=tbs[0:ncols, 0:RT])
                    if ncols < P:
                        nc.vector.memset(adst1[:, b, :], 0.0)
                    nc.vector.tensor_copy(out=adst1[0:ncols, b, :],
                                          in_=tbs[0:ncols, HID + H:HID + 2 * H])

            def allgather1():
                if no_collectives:
                    nc.sync.dma_start(out=table1[0:npc, :], in_=ag1_in[:])
                else:
                    nc.gpsimd.collective_compute(
                        "AllGather", mybir.AluOpType.bypass,
                        replica_groups=rgroups,
                        ins=[ag1_in[:].opt()], outs=[table1[:].opt()])

            # ---- edge phase (shared by both layers) ----
            def edge_phase(table_full, adst_all, t_idxB, flush_fn):
                qn = [0]
                for g in range(NB):
                    ua, ub = int(UAg[g]), int(UBg[g])
                    ut = ua + ub
                    oa = int(cfg.offA[g]) * P * 8
                    ot = int(cfg.offT[g]) * P
                    ia = idxp.tile([P, ua * 8], I16, name="ia")
                    nc.sync.dma_start(
                        out=ia[:],
                        in_=t_idxA[oa:oa + ua * P * 8].rearrange(
                            "(p k) -> p k", p=P))
                    CH = ch  # subtiles per dma_gather (ucode ring: <=1024 idxs)

                    def chunked_gather(dst_tile, src_ap, idx_tile, u, elem):
                        for c0 in range(0, u, CH):
                            c1 = min(c0 + CH, u)
                            n = (c1 - c0) * P
                            qn[0] = (qn[0] + 1) % sq
                            nc.gpsimd.dma_gather(
                                dst_tile[:, c0:c1, :], src_ap,
                                idx_tile[:, c0 * 8:c1 * 8], n, n, elem,
                                queue_num=qn[0])

                    gA = gwp.tile([P, ua, RTB], BF16, name="gA")
                    chunked_gather(gA, table_full[0:NA_ROWS, :], ia, ua, RTB)
                    tiles = [(gA, ua)]
                    if cfg.split and ub:
                        ob = int(cfg.offB[g]) * P * 8
                        ib = idxp.tile([P, ub * 8], I16, name="ib")
                        nc.sync.dma_start(
                            out=ib[:],
                            in_=t_idxB[ob:ob + ub * P * 8].rearrange(
                                "(p k) -> p k", p=P))
                        gB = gwp.tile([P, ub, RTB], BF16, name="gB")
                        chunked_gather(gB, table_full[SPLIT_AT:N, :], ib, ub, RTB)
                        tiles.append((gB, ub))
                    # host-precomputed one-hot scatter matrices (fp8):
                    # St[p, j, d] = 1 iff edge slot (p,j) targets local dst d
                    St = sp.tile([P, ut, P], F8, name="St")
                    nc.sync.dma_start(
                        out=St[:],
                        in_=t_st[:, ot:ot + ut * P].rearrange(
                            "p (k d) -> p k d", d=P))
                    StT = stp.tile([P, ut, P], F8, name="StT")
                    nc.sync.dma_start(
                        out=StT[:],
                        in_=t_stt[:, ot:ot + ut * P].rearrange(
                            "p (k d) -> p k d", d=P))
                    pad = ps_ad.tile([P, ut, H], F32, name="pad")
                    for j in range(ut):
                        nc.tensor.matmul(pad[:, j, :], StT[:, j, :],
                                         adst_all[:, g, :],
                                         start=True, stop=True)
                    # e = exp(lrelu(asrc + adst)); w = h * e
                    padb = zzp.tile([P, ut, H], BF16, name="padb")
                    nc.scalar.activation(padb[:], pad[:],
                                         mybir.ActivationFunctionType.Copy)
                    zt = zzp.tile([P, ut, H], BF16, name="zt")
                    off = 0
                    for (gt, u) in tiles:
                        nc.vector.tensor_add(
                            out=zt[:, off:off + u, :],
                            in0=gt[:, :, HID:GC],
                            in1=padb[:, off:off + u, :])
                        off += u
                    nc.vector.scalar_tensor_tensor(
                        out=zt[:], in0=zt[:], scalar=0.2, in1=zt[:],
                        op0=mybir.AluOpType.mult, op1=mybir.AluOpType.max)
                    off = 0
                    for (gt, u) in tiles:
                        nc.scalar.activation(gt[:, :, HID:GC],
                                             zt[:, off:off + u, :],
                                             mybir.ActivationFunctionType.Exp)
                        e_b = gt[:, :, HID:GC].to_broadcast([P, u, H, cfg.C])
                        hv = gt[:, :, 0:HID].rearrange(
                            "p k (h c) -> p k h c", c=cfg.C)
                        nc.vector.tensor_mul(out=hv, in0=hv, in1=e_b)
                        off += u
                    acc = ps_acc.tile([P, GC], F32, name="acc")
                    for j in range(ut):
                        gt, k = (gA, j) if j < ua else (gB, j - ua)
                        lh = St[:, j, :]
                        rh = gt[:, k, 0:GC]
                        if f32r:
                            lh, rh = lh.bitcast(F32R), rh.bitcast(F32R)
                        nc.tensor.matmul(acc[:], lh, rh,
                                         start=(j == 0),
                                         stop=(j == ut - 1))
                    flush_fn(g, acc)

            # ---- flush helpers ----
            def normalize(acc, bias_sb):
                rec = zzp.tile([P, H], F32, name="rec")
                nc.vector.reciprocal(rec[:], acc[:, HID:GC])
                xb = xbp.tile([P, HID], F32, name="xb")
                nc.vector.tensor_mul(
                    out=xb[:].rearrange("p (h c) -> p h c", c=cfg.C),
                    in0=acc[:, 0:HID].rearrange("p (h c) -> p h c", c=cfg.C),
                    in1=rec[:].to_broadcast([P, H, cfg.C]))
                nc.vector.tensor_add(out=xb[:], in0=xb[:], in1=bias_sb[:])
                nc.vector.tensor_scalar_max(xb[:], xb[:], 0.0)
                return xb

            def transpose2(xb):
                outs = []
                for a in range(HC):
                    pst = ps_tp.tile([P, P], F32, name="pst")
                    nc.tensor.transpose(pst[:], xb[:, a * P:(a + 1) * P],
                                        ident[:])
                    xts = xtbp.tile([P, P], F32, name="xts")
                    nc.scalar.activation(xts[:], pst[:],
                                         mybir.ActivationFunctionType.Copy)
                    outs.append(xts)
                return outs

            def flush_layer1(g, acc):
                ng = min(P, npc - g * P)
                xb = normalize(acc, b1sb)
                xts = transpose2(xb)
                pstb = ps_tb.tile([P, RT], F32, name="pstb2", tag="pstb")
                for a in range(HC):
                    lh, rh = xts[a][:, 0:ng], M2sb[:, a, :]
                    if f32r:
                        lh, rh = lh.bitcast(F32R), rh.bitcast(F32R)
                    nc.tensor.matmul(pstb[0:ng, :], lh, rh, start=(a == 0),
                                     stop=(a == HC - 1))
                tbs = tblp.tile([P, RTB], BF16, name="tbs2")
                nc.scalar.activation(tbs[0:ng, 0:RT], pstb[0:ng, :],
                                     mybir.ActivationFunctionType.Copy)
                nc.sync.dma_start(out=ag2_in[g * P:g * P + ng, 0:RT],
                                  in_=tbs[0:ng, 0:RT])
                if ng < P:
                    nc.vector.memset(adst2[:, g, :], 0.0)
                nc.vector.tensor_copy(out=adst2[0:ng, g, :],
                                      in_=tbs[0:ng, HID + H:HID + 2 * H])

            def flush_layer2(g, acc):
                ng = min(P, npc - g * P)
                xb = normalize(acc, b2sb)
                xts = transpose2(xb)
                pslg = ps_tb.tile([P, OUT], F32, name="pslg", tag="pstb")
                for a in range(HC):
                    nc.tensor.matmul(pslg[0:ng, :], xts[a][:, 0:ng],
                                     WcSb[:, a, :], start=(a == 0),
                                     stop=(a == HC - 1))
                lg = hdp.tile([P, OUT], F32, name="lg")
                nc.vector.tensor_add(out=lg[0:ng, :], in0=pslg[0:ng, :],
                                     in1=bcsb[0:ng, :])
                mx = hdp.tile([P, 1], F32, name="mx")
                nc.vector.tensor_reduce(out=mx[0:ng, :], in_=lg[0:ng, :],
                                        axis=mybir.AxisListType.X,
                                        op=mybir.AluOpType.max)
                nc.vector.tensor_sub(out=lg[0:ng, :], in0=lg[0:ng, :],
                                     in1=mx[0:ng, :].to_broadcast([ng, OUT]))
                ex = hdp.tile([P, OUT], F32, name="ex")
                dn = hdp.tile([P, 1], F32, name="dn")
                nc.scalar.activation(ex[0:ng, :], lg[0:ng, :],
                                     mybir.ActivationFunctionType.Exp,
                                     accum_out=dn[0:ng, :])
                lnd = hdp.tile([P, 1], F32, name="lnd")
                nc.scalar.activation(lnd[0:ng, :], dn[0:ng, :],
                                     mybir.ActivationFunctionType.Ln)
                ob = hdp.tile([P, OUT], F32, name="ob")
                nc.vector.tensor_sub(out=ob[0:ng, :], in0=lg[0:ng, :],
                                     in1=lnd[0:ng, :].to_broadcast([ng, OUT]))
                nc.sync.dma_start(out=t_out[g * P:g * P + ng, :],
                                  in_=ob[0:ng, :])

            for _wr in range(whole_reps):
                phase_b()
                allgather1()
                for _ in range(edge_reps):
                    edge_phase(table1, adst1, t_idxB if cfg.split else None,
                               flush_layer1)
                if no_collectives:
                    nc.sync.dma_start(out=table2[0:npc, :], in_=ag2_in[:])
                else:
                    nc.gpsimd.collective_compute(
                        "AllGather", mybir.AluOpType.bypass,
                        replica_groups=rgroups,
                        ins=[ag2_in[:].opt()], outs=[table2[:].opt()])
                for _ in range(edge_reps):
                    edge_phase(table2, adst2, t_idxB if cfg.split else None,
                               flush_layer2)

    nc.compile()
    return nc


def make_in_maps(cfg: Cfg, pre, x, W1, as1, ad1, b1, W2, as2, ad2, b2, Wc, bc):
    H, C, HID, npc, RT = cfg.H, cfg.C, cfg.HID, cfg.npc, cfg.RT

    def mk_m(W, a_s, a_d):
        M = np.zeros((W.shape[0], RT), np.float32)
        M[:, 0:HID] = W
        M[:, HID:HID + H] = W @ expand_att(a_s, HID, H, C)
        M[:, HID + H:HID + 2 * H] = W @ expand_att(a_d, HID, H, C)
        return M

    import ml_dtypes
    F8NP = mybir.dt.np(mybir.dt.float8e4)
    M1 = mk_m(W1, as1, ad1)
    M2 = mk_m(W2, as2, ad2)
    maps = []
    for c in range(cfg.n_cores):
        m = {
            "xT": np.ascontiguousarray(x[c * npc:(c + 1) * npc].T,
                                       dtype=np.float32),
            "M1": M1, "M2": M2, "Wc": Wc.astype(np.float32),
            "b1": np.tile(b1[None, :], (P, 1)).astype(np.float32),
            "b2": np.tile(b2[None, :], (P, 1)).astype(np.float32),
            "bc": np.tile(bc[None, :], (P, 1)).astype(np.float32),
            "idxA": pre[c]["idxA"],
            "st": pre[c]["st"].view(F8NP),
            "stt": pre[c]["stt"].view(F8NP),
        }
        if cfg.split:
            m["idxB"] = pre[c]["idxB"]
        maps.append(m)
    return maps


BUILD_KW = dict(gw_bufs=3, idx_bufs=4, sq=4)


# ---------------------------------------------------------------------------
# Harness entry point: full inputs in, full output out.
# ---------------------------------------------------------------------------

def kernel(x, edge_index, W1, as1, ad1, b1, W2, as2, ad2, b2, Wc, bc):
    x = np.asarray(x, dtype=np.float32)
    edge_index = np.asarray(edge_index)
    N, DIN = x.shape
    H, C = np.asarray(as1).shape
    OUT = np.asarray(Wc).shape[1]
    n_cores = 8

    cfg = Cfg(N, DIN, H, C, OUT, n_cores)
    pre = preprocess(cfg, edge_index)
    nc = build_program(cfg, **BUILD_KW)
    in_maps = make_in_maps(cfg, pre, x,
                           np.asarray(W1, np.float32), np.asarray(as1, np.float32),
                           np.asarray(ad1, np.float32), np.asarray(b1, np.float32),
                           np.asarray(W2, np.float32), np.asarray(as2, np.float32),
                           np.asarray(ad2, np.float32), np.asarray(b2, np.float32),
                           np.asarray(Wc, np.float32), np.asarray(bc, np.float32))

    from concourse import bass_utils
    last_err = None
    for _attempt in range(3):   # a wedged device from a prior crash can fail once
        try:
            res = bass_utils.run_bass_kernel_spmd(nc, in_maps,
                                                  core_ids=list(range(n_cores)))
            break
        except Exception as e:                      # noqa: BLE001
            last_err = e
    else:
        raise last_err
    return np.concatenate([res.results[c]["out"] for c in range(n_cores)],
                          axis=0)



# revision 47
# speedup vs baseline: 1.3931x; 1.3931x over previous
"""Distributed GAT (2-layer, PyG GATConv semantics) as a Bass/Tile SPMD kernel
for 8 Trainium2 NeuronCores — v3.

Nodes are sharded across the 8 cores (graph parallel); each layer builds the
local slab of the node table (h | alpha_src | alpha_dst, bf16 768B rows),
AllGathers it into a device-Shared DRAM table, then a dst-grouped edge phase
gathers per-edge source rows (dma_gather, 4 SWDGE queues) and scatter-sums
them into per-dst-group PSUM accumulators via one-hot matmuls.

Changes vs v2:
  - The DVE-built one-hot scatter matrices (is_equal + StreamTranspose, the
    v2 DVE bottleneck at ~75% busy) are replaced by host-precomputed fp8 St /
    StT slabs streamed from DRAM (fp8 lhsT x bf16 rhs matmuls are legal).
  - AllGather outputs are device-Shared: each core writes its 4.8MB slab once
    instead of receiving the full 38MB table (the two AllGathers drop from
    ~0.6ms to ~0.25ms).
  - Self loops never enter the gather stream: the self contribution is folded
    into normalize from an on-chip copy of the local slab (hs1/hs2), saving
    ~10% of gathered rows and the padding they forced.
  - The whole wrapped-idx slab (13.5KB/partition) and the transposed input
    x^T (bf16) stay resident in SBUF; St/StT and idx loads are merged to one
    DMA each per group, with St/StT issued from the ACT queue (the SP
    sequencer was saturated dispatching ~700 DMAs).
  - Weight stack (M1/M2/Wc + transposed activations) in bf16; PSUM pools
    rebalanced (3 acc banks) and normalize evacuates the accumulator through
    an ACT copy so the bank frees early; the self-loop prep runs at group
    start, off the critical chain.
At this point the kernel is close to the aggregate-HBM roofline for its byte
layout (~1.9GB total traffic across the 8 cores per invocation): gathers
2x83MB + St/StT 2x27MB per core, AllGathers, tables. fp8 gathered h (512B
rows) was tried and reverted: the quantization alone costs ~1e-2 final
relative error (2.3e-2 measured on HW), too close to the 2e-2 gate.
"""
import math
import numpy as np

import concourse.bass as bass
import concourse.bacc as bacc
import concourse.tile as tile
from concourse import mybir
from concourse.masks import make_identity

F32 = mybir.dt.float32
F32R = mybir.dt.float32r
BF16 = mybir.dt.bfloat16
F8 = mybir.dt.float8e4
I16 = mybir.dt.int16

P = 128
SPLIT_AT = 32768          # int16 index limit for dma_gather


class Cfg:
    def __init__(self, N, DIN, H, C, OUT, n_cores):
        self.N, self.DIN, self.H, self.C, self.OUT = N, DIN, H, C, OUT
        self.HID = H * C
        self.GC = self.HID + H              # useful gathered cols: h | asrc
        self.RT = 320                       # table-build psum row (f32)
        self.RTB = 384                      # bf16 table row (768B % 256 == 0)
        assert self.HID + 2 * H <= self.RT
        self.n_cores = n_cores
        assert N % n_cores == 0
        self.npc = N // n_cores
        assert self.npc <= SPLIT_AT, "local dst must fit int16"
        self.NB = math.ceil(self.npc / P)
        self.npc_pad = self.NB * P
        self.split = N > SPLIT_AT
        # filled by preprocess: per-group subtile counts (uniform over cores)
        self.UAg = None     # [NB] int
        self.UBg = None     # [NB] int
        self.offA = None    # [NB] slab offsets (subtile units)
        self.offB = None
        self.offT = None


def _wrap_idx(vals):
    """int16 index list (len % 16 == 0) -> dma_gather wrapped layout
    [128, len/16]: index j at partition j%16 col j//16, replicated x8."""
    n = len(vals)
    w = vals.reshape(n // 16, 16).T.astype(np.int16)   # [16, n/16]
    return np.tile(w, (8, 1))                          # [128, n/16]


def preprocess(cfg: Cfg, edge_index: np.ndarray):
    """Per-core edge-stream arrays for the group-wise dma_gathers.

    Returns list per core of dict (flat ragged slabs, one per group):
      idxA [P * sum(UAg) * 8] i16   (src < SPLIT_AT)
      idxB [P * sum(UBg) * 8] i16   (src - SPLIT_AT)   (only if cfg.split)
      st   [P, sum(UTg) * P] fp8    one-hot scatter matrices, partition-major:
        st[p, (offT[g]+j)*P + d] = 1 iff edge slot (p, j) of group g targets
        group-local dst d; all-zero row for padding slots.
      stt  [P, sum(UTg) * P] fp8    per-subtile transposes:
        stt[d, (offT[g]+j)*P + p] = st[p, (offT[g]+j)*P + d]
    """
    N, n_cores, npc = cfg.N, cfg.n_cores, cfg.npc
    NB, npc_pad = cfg.NB, cfg.npc_pad

    # self loops are handled locally in normalize (from the core's own table
    # slab), so only the real edges go through the gather stream
    src = np.asarray(edge_index[0], dtype=np.int64)
    dst = np.asarray(edge_index[1], dtype=np.int64)
    order = np.argsort(dst, kind="stable")
    src_s = src[order]
    dst_s = dst[order]
    bounds = np.searchsorted(dst_s, np.arange(n_cores + 1) * npc)

    cores = []
    cntA_all = np.zeros((n_cores, NB), np.int64)
    cntB_all = np.zeros((n_cores, NB), np.int64)
    for c in range(n_cores):
        lo, hi = bounds[c], bounds[c + 1]
        s_c = src_s[lo:hi]
        d_c = dst_s[lo:hi] - c * npc
        isB = (s_c >= SPLIT_AT) if cfg.split else np.zeros(len(s_c), bool)
        g_c = d_c // P
        # sort by (group, section, dst)
        key = (g_c * 2 + isB) * npc_pad + d_c
        o = np.argsort(key, kind="stable")
        s_c, d_c, g_c, isB = s_c[o], d_c[o], g_c[o], isB[o]
        cntA_all[c] = np.bincount(g_c[~isB], minlength=NB)
        cntB_all[c] = np.bincount(g_c[isB], minlength=NB)
        cores.append((s_c, d_c, g_c, isB))

    UAg = np.maximum(np.ceil(cntA_all.max(axis=0) / P).astype(np.int64), 1)
    if cfg.split:
        UBg = np.maximum(np.ceil(cntB_all.max(axis=0) / P).astype(np.int64), 1)
    else:
        UBg = np.zeros(NB, np.int64)
    cfg.UAg, cfg.UBg = UAg, UBg
    UTg = UAg + UBg
    cfg.offA = np.concatenate([[0], np.cumsum(UAg)[:-1]])
    cfg.offB = np.concatenate([[0], np.cumsum(UBg)[:-1]])
    cfg.offT = np.concatenate([[0], np.cumsum(UTg)[:-1]])

    # stream slot bases per group
    baseT = np.zeros(NB, np.int64)
    baseT[1:] = np.cumsum(UTg * P)[:-1]
    total_slots = int((UTg * P).sum())

    out = []
    for c, (s_c, d_c, g_c, isB) in enumerate(cores):
        cntA = cntA_all[c]
        cntB = cntB_all[c]
        startA = np.zeros(NB + 1, np.int64)
        np.cumsum(cntA, out=startA[1:])
        startB = np.zeros(NB + 1, np.int64)
        np.cumsum(cntB, out=startB[1:])
        rank = np.empty(len(d_c), np.int64)
        idxall = np.arange(len(d_c), dtype=np.int64)
        secA = ~isB
        # edges sorted by (group, section, dst): rank within own section
        rank[secA] = idxall[secA] - (startA[g_c[secA]] + startB[g_c[secA]])
        rank[isB] = idxall[isB] - (startA[g_c[isB] + 1] + startB[g_c[isB]])
        tgt = baseT[g_c] + np.where(isB, UAg[g_c] * P + rank, rank)

        srcv = np.zeros(total_slots, np.int64)          # pad -> row 0
        dstcv = np.full(total_slots, -1.0, np.float32)  # pad -> no dst
        srcv[tgt] = np.where(isB, s_c - SPLIT_AT, s_c)
        dstcv[tgt] = (d_c % P).astype(np.float32)

        sumUT = int(UTg.sum())
        # combined wrapped idx slab: per group [P, ua*8 | ub*8]
        idxs = np.zeros((P, sumUT * 8), np.int16)
        # one-hot scatter matrices as raw fp8e4 bit patterns (1.0 = 0x38),
        # St and StT interleaved per subtile: [P, sumUT, 2, P]
        sts = np.zeros((P, sumUT, 2, P), np.uint8)
        for g in range(NB):
            ua, ub, ut = int(UAg[g]), int(UBg[g]), int(UTg[g])
            sl = srcv[baseT[g]:baseT[g] + ut * P]
            dl = dstcv[baseT[g]:baseT[g] + ut * P]
            ot = int(cfg.offT[g])
            idxs[:, ot * 8:(ot + ua) * 8] = _wrap_idx(sl[:ua * P])
            if ub:
                idxs[:, (ot + ua) * 8:(ot + ut) * 8] = _wrap_idx(sl[ua * P:])
            # dc[p, j] = group-local dst of edge slot (p, j), -1 for padding
            dc = dl.reshape(ut, P).T.astype(np.int64)     # [P, ut]
            onehot = (dc[:, :, None] == np.arange(P)[None, None, :])
            sts[:, ot:ot + ut, 0, :] = onehot * np.uint8(0x38)
            sts[:, ot:ot + ut, 1, :] = onehot.transpose(2, 1, 0) * np.uint8(0x38)
        d = {"idx": idxs,
             "sts": np.ascontiguousarray(sts.reshape(P, sumUT * 2 * P))}
        out.append(d)
    return out


def expand_att(a, HID, H, C):
    A = np.zeros((HID, H), np.float32)
    for h in range(H):
        A[h * C:(h + 1) * C, h] = a[h]
    return A


def build_program(cfg: Cfg, edge_reps=1, whole_reps=1, no_collectives=False,
                  pert=frozenset(), strip=None,
                  gw_bufs=3, idx_bufs=4, sq=4, f32r=False, ch=8):
    """Emit the (core-uniform) SPMD program. Returns nc."""
    NB = cfg.NB
    UAg, UBg = cfg.UAg, cfg.UBg
    RT, RTB, GC = cfg.RT, cfg.RTB, cfg.GC
    HID, OUT, DIN, H = cfg.HID, cfg.OUT, cfg.DIN, cfg.H
    npc, N = cfg.npc, cfg.N
    DC = DIN // P
    HC = HID // P
    NA_ROWS = min(N, SPLIT_AT)
    sumUT = int((UAg + UBg).sum())

    nc = bacc.Bacc("TRN2", target_bir_lowering=False, debug=False,
                   num_devices=cfg.n_cores, num_swdge_queues=sq)

    t_xT = nc.dram_tensor("xT", [DIN, npc], BF16, kind="ExternalInput")
    t_M1 = nc.dram_tensor("M1", [DIN, RT], BF16, kind="ExternalInput")
    t_M2 = nc.dram_tensor("M2", [HID, RT], BF16, kind="ExternalInput")
    t_Wc = nc.dram_tensor("Wc", [HID, OUT], BF16, kind="ExternalInput")
    t_b1 = nc.dram_tensor("b1", [P, HID], F32, kind="ExternalInput")
    t_b2 = nc.dram_tensor("b2", [P, HID], F32, kind="ExternalInput")
    t_bc = nc.dram_tensor("bc", [P, OUT], F32, kind="ExternalInput")
    t_idx = nc.dram_tensor("idx", [P, sumUT * 8], I16, kind="ExternalInput")
    t_sts = nc.dram_tensor("sts", [P, sumUT * 2 * P], F8, kind="ExternalInput")
    t_out = nc.dram_tensor("out", [npc, OUT], F32, kind="ExternalOutput")

    rgroups = [list(range(cfg.n_cores))]

    with tile.TileContext(nc) as tc:
        with (
            tc.tile_pool(name="const", bufs=1) as cp,
            tc.tile_pool(name="xt", bufs=2) as xtp,
            tc.tile_pool(name="tbl", bufs=3) as tblp,
            tc.tile_pool(name="gw", bufs=gw_bufs) as gwp,
            tc.tile_pool(name="idx", bufs=idx_bufs) as idxp,
            tc.tile_pool(name="zz", bufs=3) as zzp,
            tc.tile_pool(name="smat", bufs=gw_bufs) as sp,
            tc.tile_pool(name="stt", bufs=gw_bufs) as stp,
            tc.tile_pool(name="xb", bufs=2) as xbp,
            tc.tile_pool(name="xtb", bufs=2) as xtbp,
            tc.tile_pool(name="hd", bufs=2) as hdp,
            tc.tile_pool(name="ps_acc", bufs=3, space="PSUM") as ps_acc,
            tc.tile_pool(name="ps_tp", bufs=2, space="PSUM") as ps_tp,
            tc.tile_pool(name="ps_ad", bufs=1, space="PSUM") as ps_ad,
            tc.tile_pool(name="ps_tb", bufs=2, space="PSUM") as ps_tb,
            tc.tile_pool(name="dram", bufs=1, space="DRAM") as dp,
        ):
            # ---- constants ----
            M1sb = cp.tile([P, DC, RT], BF16)
            M2sb = cp.tile([P, HC, RT], BF16)
            WcSb = cp.tile([P, HC, OUT], BF16)
            b1sb = cp.tile([P, HID], F32)
            b2sb = cp.tile([P, HID], F32)
            bcsb = cp.tile([P, OUT], F32)
            ident = cp.tile([P, P], F32)
            # whole input (transposed, bf16) resident in SBUF
            xt_all = cp.tile([P, DC, npc], BF16)
            # whole wrapped-idx slab resident in SBUF
            idx_all = cp.tile([P, sumUT * 8], I16)
            # per-layer local table slabs [h | asrc | adst], kept on-chip for
            # self-loop handling; adst slices feed the pad matmuls
            hs1 = cp.tile([P, NB, HID + 2 * H], BF16)
            hs2 = (cp.tile([P, NB, HID + 2 * H], BF16, name="hs2")
                   if not strip else hs1)
            adst1 = hs1[:, :, HID + H:HID + 2 * H]
            adst2 = hs2[:, :, HID + H:HID + 2 * H]
            nc.sync.dma_start(out=M1sb[:], in_=t_M1[:, :].rearrange(
                "(a c) r -> c a r", c=P))
            nc.sync.dma_start(out=M2sb[:], in_=t_M2[:, :].rearrange(
                "(a c) r -> c a r", c=P))
            nc.sync.dma_start(out=WcSb[:], in_=t_Wc[:, :].rearrange(
                "(a c) r -> c a r", c=P))
            nc.sync.dma_start(out=b1sb[:], in_=t_b1[:, :])
            nc.sync.dma_start(out=b2sb[:], in_=t_b2[:, :])
            nc.sync.dma_start(out=bcsb[:], in_=t_bc[:, :])
            nc.sync.dma_start(out=xt_all[:], in_=t_xT[:, :].rearrange(
                "(a c) n -> c a n", c=P))
            nc.sync.dma_start(out=idx_all[:], in_=t_idx[:, :])
            make_identity(nc, ident[:])

            # ---- internal DRAM ----
            # npc_pad rows: the tail group's pad rows are written (zeroed tbs
            # rows) so the local-slab loads in normalize stay finite
            ag1_in = dp.tile([cfg.npc_pad, RTB], BF16)
            ag2_in = dp.tile([cfg.npc_pad, RTB], BF16)
            shared = not no_collectives
            aspace = "Shared" if shared else "Local"
            tables = [
                (nc.dram_tensor(f"table1r{_wr}", [N, RTB], BF16,
                                addr_space=aspace)[:, :],
                 nc.dram_tensor(f"table2r{_wr}", [N, RTB], BF16,
                                addr_space=aspace)[:, :])
                for _wr in range(whole_reps)
            ]

            # ---- phase B: layer-1 table ----
            def phase_b():
                for b in range(NB):
                    ncols = min(P, npc - b * P)
                    pstb = ps_tb.tile([P, RT], F32, name="pstb", tag="pstb")
                    for a in range(DC):
                        lh = xt_all[:, a, b * P:b * P + ncols]
                        rh = M1sb[:, a, :]
                        nc.tensor.matmul(pstb[0:ncols, :], lh, rh,
                                         start=(a == 0), stop=(a == DC - 1))
                    tbs = tblp.tile([P, HID + 2 * H], BF16, name="tbs")
                    if ncols < P:
                        nc.vector.memset(tbs[:], 0.0)
                    nc.scalar.activation(tbs[0:ncols, :],
                                         pstb[0:ncols, 0:HID + 2 * H],
                                         mybir.ActivationFunctionType.Copy)
                    nc.sync.dma_start(
                        out=ag1_in[b * P:(b + 1) * P, 0:HID + 2 * H],
                        in_=tbs[:])
                    nc.vector.tensor_copy(out=hs1[:, b, :], in_=tbs[:])

            AGC = HID + 2 * H
            def allgather1(table1):
                if no_collectives:
                    nc.sync.dma_start(out=table1[0:npc, 0:AGC],
                                      in_=ag1_in[0:npc, 0:AGC])
                else:
                    nc.gpsimd.collective_compute(
                        "AllGather", mybir.AluOpType.bypass,
                        replica_groups=rgroups,
                        ins=[ag1_in[0:npc, :].opt()], outs=[table1.opt()])

            # ---- edge phase (shared by both layers) ----
            def edge_phase(table_full, adst_all, flush_fn, hs):
                qn = [0]
                for g in range(NB):
                    ua, ub = int(UAg[g]), int(UBg[g])
                    ut = ua + ub
                    ot = int(cfg.offT[g])
                    pre = None if strip == "gather" else normalize_pre(g, hs)
                    iab = idx_all[:, ot * 8:(ot + ut) * 8]
                    ia = iab[:, 0:ua * 8]
                    CH = ch  # subtiles per dma_gather (ucode ring: <=1024 idxs)

                    def chunked_gather(dst_tile, src_ap, idx_tile, u, elem):
                        for c0 in range(0, u, CH):
                            c1 = min(c0 + CH, u)
                            n = (c1 - c0) * P
                            qn[0] = (qn[0] + 1) % sq
                            nc.gpsimd.dma_gather(
                                dst_tile[:, c0:c1, :], src_ap,
                                idx_tile[:, c0 * 8:c1 * 8], n, n, elem,
                                queue_num=qn[0])

                    gA = gwp.tile([P, ua, RTB], BF16, name="gA")
                    chunked_gather(gA, table_full[0:NA_ROWS, :], ia, ua, RTB)
                    tiles = [(gA, ua)]
                    if cfg.split and ub:
                        ib = iab[:, ua * 8:ut * 8]
                        gB = gwp.tile([P, ub, RTB], BF16, name="gB")
                        chunked_gather(gB, table_full[SPLIT_AT:N, :], ib, ub, RTB)
                        tiles.append((gB, ub))
                    if strip == "gather":
                        continue
                    # host-precomputed one-hot scatter matrices (fp8),
                    # St/StT interleaved per subtile, ACT-issued DMA:
                    # St[p, j, d] = 1 iff edge slot (p,j) targets local dst d
                    sts_t = sp.tile([P, ut, 2, P], F8, name="sts")
                    nc.scalar.dma_start(
                        out=sts_t[:],
                        in_=t_sts[:, ot * 2 * P:(ot + ut) * 2 * P].rearrange(
                            "p (k h d) -> p k h d", h=2, d=P))
                    St = sts_t[:, :, 0, :]
                    StT = sts_t[:, :, 1, :]
                    pad = ps_ad.tile([P, ut, H], F32, name="pad")
                    for j in range(ut):
                        nc.tensor.matmul(pad[:, j, :], StT[:, j, :],
                                         adst_all[:, g, :],
                                         start=True, stop=True)
                    # e = exp(lrelu(asrc + adst)); w = h * e
                    padb = zzp.tile([P, ut, H], BF16, name="padb")
                    nc.scalar.activation(padb[:], pad[:],
                                         mybir.ActivationFunctionType.Copy)
                    zt = zzp.tile([P, ut, H], BF16, name="zt")
                    off = 0
                    for (gt, u) in tiles:
                        nc.vector.tensor_add(
                            out=zt[:, off:off + u, :],
                            in0=gt[:, :, HID:GC],
                            in1=padb[:, off:off + u, :])
                        off += u
                    nc.vector.scalar_tensor_tensor(
                        out=zt[:], in0=zt[:], scalar=0.2, in1=zt[:],
                        op0=mybir.AluOpType.mult, op1=mybir.AluOpType.max)
                    off = 0
                    for (gt, u) in tiles:
                        nc.scalar.activation(gt[:, :, HID:GC],
                                             zt[:, off:off + u, :],
                                             mybir.ActivationFunctionType.Exp)
                        e_b = gt[:, :, HID:GC].to_broadcast([P, u, H, cfg.C])
                        hv = gt[:, :, 0:HID].rearrange(
                            "p k (h c) -> p k h c", c=cfg.C)
                        nc.vector.tensor_mul(out=hv, in0=hv, in1=e_b)
                        off += u
                    if strip == "noacc":
                        continue
                    acc = ps_acc.tile([P, GC], F32, name="acc")
                    for j in range(ut):
                        gt, k = (gA, j) if j < ua else (gB, j - ua)
                        lh = St[:, j, :]
                        rh = gt[:, k, 0:GC]
                        nc.tensor.matmul(acc[:], lh, rh,
                                         start=(j == 0),
                                         stop=(j == ut - 1))
                    if strip == "noflush":
                        continue
                    flush_fn(g, acc, pre)

            # ---- flush helpers ----
            def normalize_pre(g, hs):
                # self-loop contribution from the on-chip local table slab
                ls = hs[:, g, :]
                asf = zzp.tile([P, H], BF16, name="asf")
                nc.vector.tensor_add(out=asf[:], in0=ls[:, HID:HID + H],
                                     in1=ls[:, HID + H:HID + 2 * H])
                nc.vector.scalar_tensor_tensor(
                    out=asf[:], in0=asf[:], scalar=0.2, in1=asf[:],
                    op0=mybir.AluOpType.mult, op1=mybir.AluOpType.max)
                es = zzp.tile([P, H], F32, name="es")
                nc.scalar.activation(es[:], asf[:],
                                     mybir.ActivationFunctionType.Exp)
                return ls, es

            def normalize(g, acc, bias_sb, pre):
                ls, es = pre
                # evacuate PSUM acc immediately so the bank frees early
                accs = xbp.tile([P, GC], F32, name="accs")
                nc.scalar.activation(accs[:], acc[:],
                                     mybir.ActivationFunctionType.Copy)
                den = zzp.tile([P, H], F32, name="den")
                nc.vector.tensor_add(out=den[:], in0=accs[:, HID:GC], in1=es[:])
                rec = zzp.tile([P, H], F32, name="rec")
                nc.vector.reciprocal(rec[:], den[:])
                xb = xbp.tile([P, HID], F32, name="xb")
                nc.vector.tensor_mul(
                    out=xb[:].rearrange("p (h c) -> p h c", c=cfg.C),
                    in0=ls[:, 0:HID].rearrange("p (h c) -> p h c", c=cfg.C),
                    in1=es[:].to_broadcast([P, H, cfg.C]))
                nc.vector.tensor_add(out=xb[:], in0=xb[:], in1=accs[:, 0:HID])
                nc.vector.tensor_mul(
                    out=xb[:].rearrange("p (h c) -> p h c", c=cfg.C),
                    in0=xb[:].rearrange("p (h c) -> p h c", c=cfg.C),
                    in1=rec[:].to_broadcast([P, H, cfg.C]))
                nc.vector.tensor_add(out=xb[:], in0=xb[:], in1=bias_sb[:])
                nc.vector.tensor_scalar_max(xb[:], xb[:], 0.0)
                return xb

            def transpose2(xb):
                outs = []
                for a in range(HC):
                    pst = ps_tp.tile([P, P], F32, name="pst")
                    nc.tensor.transpose(pst[:], xb[:, a * P:(a + 1) * P],
                                        ident[:])
                    xts = xtbp.tile([P, P], BF16, name="xts")
                    nc.scalar.activation(xts[:], pst[:],
                                         mybir.ActivationFunctionType.Copy)
                    outs.append(xts)
                return outs

            def flush_layer1(g, acc, pre):
                ng = min(P, npc - g * P)
                xb = normalize(g, acc, b1sb, pre)
                xts = transpose2(xb)
                pstb = ps_tb.tile([P, RT], F32, name="pstb2", tag="pstb")
                for a in range(HC):
                    lh, rh = xts[a][:, 0:ng], M2sb[:, a, :]
                    if f32r:
                        lh, rh = lh.bitcast(F32R), rh.bitcast(F32R)
                    nc.tensor.matmul(pstb[0:ng, :], lh, rh, start=(a == 0),
                                     stop=(a == HC - 1))
                tbs = tblp.tile([P, HID + 2 * H], BF16, name="tbs2")
                if ng < P:
                    nc.vector.memset(tbs[:], 0.0)
                nc.scalar.activation(tbs[0:ng, :], pstb[0:ng, 0:HID + 2 * H],
                                     mybir.ActivationFunctionType.Copy)
                nc.sync.dma_start(
                    out=ag2_in[g * P:(g + 1) * P, 0:HID + 2 * H],
                    in_=tbs[:])
                nc.vector.tensor_copy(out=hs2[:, g, :], in_=tbs[:])

            def flush_layer2(g, acc, pre):
                ng = min(P, npc - g * P)
                xb = normalize(g, acc, b2sb, pre)
                xts = transpose2(xb)
                pslg = ps_tb.tile([P, OUT], F32, name="pslg", tag="pstb")
                for a in range(HC):
                    nc.tensor.matmul(pslg[0:ng, :], xts[a][:, 0:ng],
                                     WcSb[:, a, :], start=(a == 0),
                                     stop=(a == HC - 1))
                lg = hdp.tile([P, OUT], F32, name="lg")
                nc.vector.tensor_add(out=lg[0:ng, :], in0=pslg[0:ng, :],
                                     in1=bcsb[0:ng, :])
                mx = hdp.tile([P, 1], F32, name="mx")
                nc.vector.tensor_reduce(out=mx[0:ng, :], in_=lg[0:ng, :],
                                        axis=mybir.AxisListType.X,
                                        op=mybir.AluOpType.max)
                nc.vector.tensor_sub(out=lg[0:ng, :], in0=lg[0:ng, :],
                                     in1=mx[0:ng, :].to_broadcast([ng, OUT]))
                ex = hdp.tile([P, OUT], F32, name="ex")
                dn = hdp.tile([P, 1], F32, name="dn")
                nc.scalar.activation(ex[0:ng, :], lg[0:ng, :],
                                     mybir.ActivationFunctionType.Exp,
                                     accum_out=dn[0:ng, :])
                lnd = hdp.tile([P, 1], F32, name="lnd")
                nc.scalar.activation(lnd[0:ng, :], dn[0:ng, :],
                                     mybir.ActivationFunctionType.Ln)
                ob = hdp.tile([P, OUT], F32, name="ob")
                nc.vector.tensor_sub(out=ob[0:ng, :], in0=lg[0:ng, :],
                                     in1=lnd[0:ng, :].to_broadcast([ng, OUT]))
                nc.sync.dma_start(out=t_out[g * P:g * P + ng, :],
                                  in_=ob[0:ng, :])

            for _wr in range(whole_reps):
                table1, table2 = tables[_wr]
                phase_b()
                allgather1(table1)
                for _ in range(edge_reps):
                    edge_phase(table1, adst1, flush_layer1, hs1)
                if no_collectives:
                    nc.sync.dma_start(out=table2[0:npc, 0:AGC],
                                      in_=ag2_in[0:npc, 0:AGC])
                else:
                    nc.gpsimd.collective_compute(
                        "AllGather", mybir.AluOpType.bypass,
                        replica_groups=rgroups,
                        ins=[ag2_in[0:npc, :].opt()], outs=[table2.opt()])
                for _ in range(edge_reps):
                    edge_phase(table2, adst2, flush_layer2,
                               hs1 if strip else hs2)

    nc.compile()
    return nc


def make_in_maps(cfg: Cfg, pre, x, W1, as1, ad1, b1, W2, as2, ad2, b2, Wc, bc):
    H, C, HID, npc, RT = cfg.H, cfg.C, cfg.HID, cfg.npc, cfg.RT

    def mk_m(W, a_s, a_d):
        M = np.zeros((W.shape[0], RT), np.float32)
        M[:, 0:HID] = W
        M[:, HID:HID + H] = W @ expand_att(a_s, HID, H, C)
        M[:, HID + H:HID + 2 * H] = W @ expand_att(a_d, HID, H, C)
        return M

    import ml_dtypes
    F8NP = mybir.dt.np(mybir.dt.float8e4)
    BF = ml_dtypes.bfloat16
    M1 = mk_m(W1, as1, ad1).astype(BF)
    M2 = mk_m(W2, as2, ad2).astype(BF)
    maps = []
    for c in range(cfg.n_cores):
        m = {
            "xT": np.ascontiguousarray(x[c * npc:(c + 1) * npc].T).astype(BF),
            "M1": M1, "M2": M2, "Wc": Wc.astype(BF),
            "b1": np.tile(b1[None, :], (P, 1)).astype(np.float32),
            "b2": np.tile(b2[None, :], (P, 1)).astype(np.float32),
            "bc": np.tile(bc[None, :], (P, 1)).astype(np.float32),
            "idx": pre[c]["idx"],
            "sts": pre[c]["sts"].view(F8NP),
        }
        maps.append(m)
    return maps


BUILD_KW = dict(gw_bufs=5, idx_bufs=4, sq=4)


# ---------------------------------------------------------------------------
# Harness entry point: full inputs in, full output out.
# ---------------------------------------------------------------------------

def kernel(x, edge_index, W1, as1, ad1, b1, W2, as2, ad2, b2, Wc, bc):
    x = np.asarray(x, dtype=np.float32)
    edge_index = np.asarray(edge_index)
    N, DIN = x.shape
    H, C = np.asarray(as1).shape
    OUT = np.asarray(Wc).shape[1]
    n_cores = 8

    cfg = Cfg(N, DIN, H, C, OUT, n_cores)
    pre = preprocess(cfg, edge_index)
    nc = build_program(cfg, **BUILD_KW)
    in_maps = make_in_maps(cfg, pre, x,
                           np.asarray(W1, np.float32), np.asarray(as1, np.float32),
                           np.asarray(ad1, np.float32), np.asarray(b1, np.float32),
                           np.asarray(W2, np.float32), np.asarray(as2, np.float32),
                           np.asarray(ad2, np.float32), np.asarray(b2, np.float32),
                           np.asarray(Wc, np.float32), np.asarray(bc, np.float32))

    from concourse import bass_utils
    last_err = None
    for _attempt in range(3):   # a wedged device from a prior crash can fail once
        try:
            res = bass_utils.run_bass_kernel_spmd(nc, in_maps,
                                                  core_ids=list(range(n_cores)))
            break
        except Exception as e:                      # noqa: BLE001
            last_err = e
    else:
        raise last_err
    return np.concatenate([res.results[c]["out"] for c in range(n_cores)],
                          axis=0)



# revision 50
# speedup vs baseline: 1.4606x; 1.0485x over previous
"""Distributed GAT (2-layer, PyG GATConv semantics) as a Bass/Tile SPMD kernel
for 8 Trainium2 NeuronCores — v3.

Nodes are sharded across the 8 cores (graph parallel); each layer builds the
local slab of the node table (h | alpha_src | alpha_dst, bf16 768B rows),
AllGathers it into a device-Shared DRAM table, then a dst-grouped edge phase
gathers per-edge source rows (dma_gather, 4 SWDGE queues) and scatter-sums
them into per-dst-group PSUM accumulators via one-hot matmuls.

Changes vs v2:
  - The DVE-built one-hot scatter matrices (is_equal + StreamTranspose, the
    v2 DVE bottleneck at ~75% busy) are replaced by host-precomputed fp8 St /
    StT slabs streamed from DRAM (fp8 lhsT x bf16 rhs matmuls are legal).
  - AllGather outputs are device-Shared: each core writes its 4.8MB slab once
    instead of receiving the full 38MB table (the two AllGathers drop from
    ~0.6ms to ~0.25ms).
  - Self loops never enter the gather stream: the self contribution is folded
    into normalize from an on-chip copy of the local slab (hs1/hs2), saving
    ~10% of gathered rows and the padding they forced.
  - The whole wrapped-idx slab (13.5KB/partition) and the transposed input
    x^T (bf16) stay resident in SBUF; St/StT and idx loads are merged to one
    DMA each per group, with St/StT issued from the ACT queue (the SP
    sequencer was saturated dispatching ~700 DMAs).
  - Weight stack (M1/M2/Wc + transposed activations) and the normalize /
    transpose path in bf16 (2x DVE rate); PSUM pools rebalanced (3 acc banks)
    and normalize evacuates the accumulator through an ACT copy so the bank
    frees early; the self-loop prep runs at group start, off the critical
    chain.
At this point the kernel is close to the aggregate-HBM roofline for its byte
layout (~1.9GB total traffic across the 8 cores per invocation): gathers
2x83MB + St/StT 2x27MB per core, AllGathers, tables. fp8 gathered h (512B
rows) was tried and reverted: the quantization alone costs ~1e-2 final
relative error (2.3e-2 measured on HW), too close to the 2e-2 gate.
"""
import math
import numpy as np

import concourse.bass as bass
import concourse.bacc as bacc
import concourse.tile as tile
from concourse import mybir
from concourse.masks import make_identity

F32 = mybir.dt.float32
F32R = mybir.dt.float32r
BF16 = mybir.dt.bfloat16
F8 = mybir.dt.float8e4
I16 = mybir.dt.int16

P = 128
SPLIT_AT = 32768          # int16 index limit for dma_gather


class Cfg:
    def __init__(self, N, DIN, H, C, OUT, n_cores):
        self.N, self.DIN, self.H, self.C, self.OUT = N, DIN, H, C, OUT
        self.HID = H * C
        self.GC = self.HID + H              # useful gathered cols: h | asrc
        self.RT = 320                       # table-build psum row (f32)
        self.RTB = 384                      # bf16 table row (768B % 256 == 0)
        assert self.HID + 2 * H <= self.RT
        self.n_cores = n_cores
        assert N % n_cores == 0
        self.npc = N // n_cores
        assert self.npc <= SPLIT_AT, "local dst must fit int16"
        self.NB = math.ceil(self.npc / P)
        self.npc_pad = self.NB * P
        self.split = N > SPLIT_AT
        # filled by preprocess: per-group subtile counts (uniform over cores)
        self.UAg = None     # [NB] int
        self.UBg = None     # [NB] int
        self.offA = None    # [NB] slab offsets (subtile units)
        self.offB = None
        self.offT = None


def _wrap_idx(vals):
    """int16 index list (len % 16 == 0) -> dma_gather wrapped layout
    [128, len/16]: index j at partition j%16 col j//16, replicated x8."""
    n = len(vals)
    w = vals.reshape(n // 16, 16).T.astype(np.int16)   # [16, n/16]
    return np.tile(w, (8, 1))                          # [128, n/16]


def preprocess(cfg: Cfg, edge_index: np.ndarray):
    """Per-core edge-stream arrays for the group-wise dma_gathers.

    Returns list per core of dict (flat ragged slabs, one per group):
      idxA [P * sum(UAg) * 8] i16   (src < SPLIT_AT)
      idxB [P * sum(UBg) * 8] i16   (src - SPLIT_AT)   (only if cfg.split)
      st   [P, sum(UTg) * P] fp8    one-hot scatter matrices, partition-major:
        st[p, (offT[g]+j)*P + d] = 1 iff edge slot (p, j) of group g targets
        group-local dst d; all-zero row for padding slots.
      stt  [P, sum(UTg) * P] fp8    per-subtile transposes:
        stt[d, (offT[g]+j)*P + p] = st[p, (offT[g]+j)*P + d]
    """
    N, n_cores, npc = cfg.N, cfg.n_cores, cfg.npc
    NB, npc_pad = cfg.NB, cfg.npc_pad

    # self loops are handled locally in normalize (from the core's own table
    # slab), so only the real edges go through the gather stream
    src = np.asarray(edge_index[0], dtype=np.int64)
    dst = np.asarray(edge_index[1], dtype=np.int64)
    order = np.argsort(dst, kind="stable")
    src_s = src[order]
    dst_s = dst[order]
    bounds = np.searchsorted(dst_s, np.arange(n_cores + 1) * npc)

    cores = []
    cntA_all = np.zeros((n_cores, NB), np.int64)
    cntB_all = np.zeros((n_cores, NB), np.int64)
    for c in range(n_cores):
        lo, hi = bounds[c], bounds[c + 1]
        s_c = src_s[lo:hi]
        d_c = dst_s[lo:hi] - c * npc
        isB = (s_c >= SPLIT_AT) if cfg.split else np.zeros(len(s_c), bool)
        g_c = d_c // P
        # sort by (group, section, dst)
        key = (g_c * 2 + isB) * npc_pad + d_c
        o = np.argsort(key, kind="stable")
        s_c, d_c, g_c, isB = s_c[o], d_c[o], g_c[o], isB[o]
        cntA_all[c] = np.bincount(g_c[~isB], minlength=NB)
        cntB_all[c] = np.bincount(g_c[isB], minlength=NB)
        cores.append((s_c, d_c, g_c, isB))

    UAg = np.maximum(np.ceil(cntA_all.max(axis=0) / P).astype(np.int64), 1)
    if cfg.split:
        UBg = np.maximum(np.ceil(cntB_all.max(axis=0) / P).astype(np.int64), 1)
    else:
        UBg = np.zeros(NB, np.int64)
    cfg.UAg, cfg.UBg = UAg, UBg
    UTg = UAg + UBg
    cfg.offA = np.concatenate([[0], np.cumsum(UAg)[:-1]])
    cfg.offB = np.concatenate([[0], np.cumsum(UBg)[:-1]])
    cfg.offT = np.concatenate([[0], np.cumsum(UTg)[:-1]])

    # stream slot bases per group
    baseT = np.zeros(NB, np.int64)
    baseT[1:] = np.cumsum(UTg * P)[:-1]
    total_slots = int((UTg * P).sum())

    out = []
    for c, (s_c, d_c, g_c, isB) in enumerate(cores):
        cntA = cntA_all[c]
        cntB = cntB_all[c]
        startA = np.zeros(NB + 1, np.int64)
        np.cumsum(cntA, out=startA[1:])
        startB = np.zeros(NB + 1, np.int64)
        np.cumsum(cntB, out=startB[1:])
        rank = np.empty(len(d_c), np.int64)
        idxall = np.arange(len(d_c), dtype=np.int64)
        secA = ~isB
        # edges sorted by (group, section, dst): rank within own section
        rank[secA] = idxall[secA] - (startA[g_c[secA]] + startB[g_c[secA]])
        rank[isB] = idxall[isB] - (startA[g_c[isB] + 1] + startB[g_c[isB]])
        tgt = baseT[g_c] + np.where(isB, UAg[g_c] * P + rank, rank)

        srcv = np.zeros(total_slots, np.int64)          # pad -> row 0
        dstcv = np.full(total_slots, -1.0, np.float32)  # pad -> no dst
        srcv[tgt] = np.where(isB, s_c - SPLIT_AT, s_c)
        dstcv[tgt] = (d_c % P).astype(np.float32)

        sumUT = int(UTg.sum())
        # combined wrapped idx slab: per group [P, ua*8 | ub*8]
        idxs = np.zeros((P, sumUT * 8), np.int16)
        # one-hot scatter matrices as raw fp8e4 bit patterns (1.0 = 0x38),
        # St and StT interleaved per subtile: [P, sumUT, 2, P]
        sts = np.zeros((P, sumUT, 2, P), np.uint8)
        for g in range(NB):
            ua, ub, ut = int(UAg[g]), int(UBg[g]), int(UTg[g])
            sl = srcv[baseT[g]:baseT[g] + ut * P]
            dl = dstcv[baseT[g]:baseT[g] + ut * P]
            ot = int(cfg.offT[g])
            idxs[:, ot * 8:(ot + ua) * 8] = _wrap_idx(sl[:ua * P])
            if ub:
                idxs[:, (ot + ua) * 8:(ot + ut) * 8] = _wrap_idx(sl[ua * P:])
            # dc[p, j] = group-local dst of edge slot (p, j), -1 for padding
            dc = dl.reshape(ut, P).T.astype(np.int64)     # [P, ut]
            onehot = (dc[:, :, None] == np.arange(P)[None, None, :])
            sts[:, ot:ot + ut, 0, :] = onehot * np.uint8(0x38)
            sts[:, ot:ot + ut, 1, :] = onehot.transpose(2, 1, 0) * np.uint8(0x38)
        d = {"idx": idxs,
             "sts": np.ascontiguousarray(sts.reshape(P, sumUT * 2 * P))}
        out.append(d)
    return out


def expand_att(a, HID, H, C):
    A = np.zeros((HID, H), np.float32)
    for h in range(H):
        A[h * C:(h + 1) * C, h] = a[h]
    return A


def build_program(cfg: Cfg, edge_reps=1, whole_reps=1, no_collectives=False,
                  pert=frozenset(), strip=None,
                  gw_bufs=3, idx_bufs=4, sq=4, f32r=False, ch=8):
    """Emit the (core-uniform) SPMD program. Returns nc."""
    NB = cfg.NB
    UAg, UBg = cfg.UAg, cfg.UBg
    RT, RTB, GC = cfg.RT, cfg.RTB, cfg.GC
    HID, OUT, DIN, H = cfg.HID, cfg.OUT, cfg.DIN, cfg.H
    npc, N = cfg.npc, cfg.N
    DC = DIN // P
    HC = HID // P
    NA_ROWS = min(N, SPLIT_AT)
    sumUT = int((UAg + UBg).sum())

    nc = bacc.Bacc("TRN2", target_bir_lowering=False, debug=False,
                   num_devices=cfg.n_cores, num_swdge_queues=sq)

    t_xT = nc.dram_tensor("xT", [DIN, npc], BF16, kind="ExternalInput")
    t_M1 = nc.dram_tensor("M1", [DIN, RT], BF16, kind="ExternalInput")
    t_M2 = nc.dram_tensor("M2", [HID, RT], BF16, kind="ExternalInput")
    t_Wc = nc.dram_tensor("Wc", [HID, OUT], BF16, kind="ExternalInput")
    t_b1 = nc.dram_tensor("b1", [P, HID], F32, kind="ExternalInput")
    t_b2 = nc.dram_tensor("b2", [P, HID], F32, kind="ExternalInput")
    t_bc = nc.dram_tensor("bc", [P, OUT], F32, kind="ExternalInput")
    t_idx = nc.dram_tensor("idx", [P, sumUT * 8], I16, kind="ExternalInput")
    t_sts = nc.dram_tensor("sts", [P, sumUT * 2 * P], F8, kind="ExternalInput")
    t_out = nc.dram_tensor("out", [npc, OUT], F32, kind="ExternalOutput")

    rgroups = [list(range(cfg.n_cores))]

    with tile.TileContext(nc) as tc:
        with (
            tc.tile_pool(name="const", bufs=1) as cp,
            tc.tile_pool(name="xt", bufs=2) as xtp,
            tc.tile_pool(name="tbl", bufs=3) as tblp,
            tc.tile_pool(name="gw", bufs=gw_bufs) as gwp,
            tc.tile_pool(name="idx", bufs=idx_bufs) as idxp,
            tc.tile_pool(name="zz", bufs=3) as zzp,
            tc.tile_pool(name="smat", bufs=gw_bufs) as sp,
            tc.tile_pool(name="stt", bufs=gw_bufs) as stp,
            tc.tile_pool(name="xb", bufs=2) as xbp,
            tc.tile_pool(name="xtb", bufs=2) as xtbp,
            tc.tile_pool(name="hd", bufs=2) as hdp,
            tc.tile_pool(name="ps_acc", bufs=3, space="PSUM") as ps_acc,
            tc.tile_pool(name="ps_tp", bufs=2, space="PSUM") as ps_tp,
            tc.tile_pool(name="ps_ad", bufs=1, space="PSUM") as ps_ad,
            tc.tile_pool(name="ps_tb", bufs=2, space="PSUM") as ps_tb,
            tc.tile_pool(name="dram", bufs=1, space="DRAM") as dp,
        ):
            # ---- constants ----
            M1sb = cp.tile([P, DC, RT], BF16)
            M2sb = cp.tile([P, HC, RT], BF16)
            WcSb = cp.tile([P, HC, OUT], BF16)
            b1sb = cp.tile([P, HID], F32)
            b2sb = cp.tile([P, HID], F32)
            bcsb = cp.tile([P, OUT], F32)
            ident = cp.tile([P, P], F32)
            identb = cp.tile([P, P], BF16)
            # whole input (transposed, bf16) resident in SBUF
            xt_all = cp.tile([P, DC, npc], BF16)
            # whole wrapped-idx slab resident in SBUF
            idx_all = cp.tile([P, sumUT * 8], I16)
            # per-layer local table slabs [h | asrc | adst], kept on-chip for
            # self-loop handling; adst slices feed the pad matmuls
            hs1 = cp.tile([P, NB, HID + 2 * H], BF16)
            hs2 = (cp.tile([P, NB, HID + 2 * H], BF16, name="hs2")
                   if not strip else hs1)
            adst1 = hs1[:, :, HID + H:HID + 2 * H]
            adst2 = hs2[:, :, HID + H:HID + 2 * H]
            nc.sync.dma_start(out=M1sb[:], in_=t_M1[:, :].rearrange(
                "(a c) r -> c a r", c=P))
            nc.sync.dma_start(out=M2sb[:], in_=t_M2[:, :].rearrange(
                "(a c) r -> c a r", c=P))
            nc.sync.dma_start(out=WcSb[:], in_=t_Wc[:, :].rearrange(
                "(a c) r -> c a r", c=P))
            nc.sync.dma_start(out=b1sb[:], in_=t_b1[:, :])
            nc.sync.dma_start(out=b2sb[:], in_=t_b2[:, :])
            nc.sync.dma_start(out=bcsb[:], in_=t_bc[:, :])
            nc.sync.dma_start(out=xt_all[:], in_=t_xT[:, :].rearrange(
                "(a c) n -> c a n", c=P))
            nc.sync.dma_start(out=idx_all[:], in_=t_idx[:, :])
            make_identity(nc, ident[:])
            make_identity(nc, identb[:])

            # ---- internal DRAM ----
            # npc_pad rows: the tail group's pad rows are written (zeroed tbs
            # rows) so the local-slab loads in normalize stay finite
            ag1_in = dp.tile([cfg.npc_pad, RTB], BF16)
            ag2_in = dp.tile([cfg.npc_pad, RTB], BF16)
            shared = not no_collectives
            aspace = "Shared" if shared else "Local"
            tables = [
                (nc.dram_tensor(f"table1r{_wr}", [N, RTB], BF16,
                                addr_space=aspace)[:, :],
                 nc.dram_tensor(f"table2r{_wr}", [N, RTB], BF16,
                                addr_space=aspace)[:, :])
                for _wr in range(whole_reps)
            ]

            # ---- phase B: layer-1 table ----
            def phase_b():
                for b in range(NB):
                    ncols = min(P, npc - b * P)
                    pstb = ps_tb.tile([P, RT], F32, name="pstb", tag="pstb")
                    for a in range(DC):
                        lh = xt_all[:, a, b * P:b * P + ncols]
                        rh = M1sb[:, a, :]
                        nc.tensor.matmul(pstb[0:ncols, :], lh, rh,
                                         start=(a == 0), stop=(a == DC - 1))
                    tbs = tblp.tile([P, HID + 2 * H], BF16, name="tbs")
                    if ncols < P:
                        nc.vector.memset(tbs[:], 0.0)
                    nc.scalar.activation(tbs[0:ncols, :],
                                         pstb[0:ncols, 0:HID + 2 * H],
                                         mybir.ActivationFunctionType.Copy)
                    nc.sync.dma_start(
                        out=ag1_in[b * P:(b + 1) * P, 0:HID + 2 * H],
                        in_=tbs[:])
                    nc.vector.tensor_copy(out=hs1[:, b, :], in_=tbs[:])

            AGC = HID + 2 * H
            def allgather1(table1):
                if no_collectives:
                    nc.sync.dma_start(out=table1[0:npc, 0:AGC],
                                      in_=ag1_in[0:npc, 0:AGC])
                else:
                    nc.gpsimd.collective_compute(
                        "AllGather", mybir.AluOpType.bypass,
                        replica_groups=rgroups,
                        ins=[ag1_in[0:npc, :].opt()], outs=[table1.opt()])

            # ---- edge phase (shared by both layers) ----
            def edge_phase(table_full, adst_all, flush_fn, hs):
                qn = [0]
                for g in range(NB):
                    ua, ub = int(UAg[g]), int(UBg[g])
                    ut = ua + ub
                    ot = int(cfg.offT[g])
                    pre = None if strip == "gather" else normalize_pre(g, hs)
                    iab = idx_all[:, ot * 8:(ot + ut) * 8]
                    ia = iab[:, 0:ua * 8]
                    CH = ch  # subtiles per dma_gather (ucode ring: <=1024 idxs)

                    def chunked_gather(dst_tile, src_ap, idx_tile, u, elem):
                        for c0 in range(0, u, CH):
                            c1 = min(c0 + CH, u)
                            n = (c1 - c0) * P
                            qn[0] = (qn[0] + 1) % sq
                            nc.gpsimd.dma_gather(
                                dst_tile[:, c0:c1, :], src_ap,
                                idx_tile[:, c0 * 8:c1 * 8], n, n, elem,
                                queue_num=qn[0])

                    gA = gwp.tile([P, ua, RTB], BF16, name="gA")
                    chunked_gather(gA, table_full[0:NA_ROWS, :], ia, ua, RTB)
                    tiles = [(gA, ua)]
                    if cfg.split and ub:
                        ib = iab[:, ua * 8:ut * 8]
                        gB = gwp.tile([P, ub, RTB], BF16, name="gB")
                        chunked_gather(gB, table_full[SPLIT_AT:N, :], ib, ub, RTB)
                        tiles.append((gB, ub))
                    if strip == "gather":
                        continue
                    # host-precomputed one-hot scatter matrices (fp8),
                    # St/StT interleaved per subtile, ACT-issued DMA:
                    # St[p, j, d] = 1 iff edge slot (p,j) targets local dst d
                    sts_t = sp.tile([P, ut, 2, P], F8, name="sts")
                    nc.scalar.dma_start(
                        out=sts_t[:],
                        in_=t_sts[:, ot * 2 * P:(ot + ut) * 2 * P].rearrange(
                            "p (k h d) -> p k h d", h=2, d=P))
                    St = sts_t[:, :, 0, :]
                    StT = sts_t[:, :, 1, :]
                    pad = ps_ad.tile([P, ut, H], F32, name="pad")
                    for j in range(ut):
                        nc.tensor.matmul(pad[:, j, :], StT[:, j, :],
                                         adst_all[:, g, :],
                                         start=True, stop=True)
                    # e = exp(lrelu(asrc + adst)); w = h * e
                    padb = zzp.tile([P, ut, H], BF16, name="padb")
                    nc.scalar.activation(padb[:], pad[:],
                                         mybir.ActivationFunctionType.Copy)
                    zt = zzp.tile([P, ut, H], BF16, name="zt")
                    off = 0
                    for (gt, u) in tiles:
                        nc.vector.tensor_add(
                            out=zt[:, off:off + u, :],
                            in0=gt[:, :, HID:GC],
                            in1=padb[:, off:off + u, :])
                        off += u
                    nc.vector.scalar_tensor_tensor(
                        out=zt[:], in0=zt[:], scalar=0.2, in1=zt[:],
                        op0=mybir.AluOpType.mult, op1=mybir.AluOpType.max)
                    off = 0
                    for (gt, u) in tiles:
                        nc.scalar.activation(gt[:, :, HID:GC],
                                             zt[:, off:off + u, :],
                                             mybir.ActivationFunctionType.Exp)
                        e_b = gt[:, :, HID:GC].to_broadcast([P, u, H, cfg.C])
                        hv = gt[:, :, 0:HID].rearrange(
                            "p k (h c) -> p k h c", c=cfg.C)
                        nc.vector.tensor_mul(out=hv, in0=hv, in1=e_b)
                        off += u
                    if strip == "noacc":
                        continue
                    acc = ps_acc.tile([P, GC], F32, name="acc")
                    for j in range(ut):
                        gt, k = (gA, j) if j < ua else (gB, j - ua)
                        lh = St[:, j, :]
                        rh = gt[:, k, 0:GC]
                        nc.tensor.matmul(acc[:], lh, rh,
                                         start=(j == 0),
                                         stop=(j == ut - 1))
                    if strip == "noflush":
                        continue
                    flush_fn(g, acc, pre)

            # ---- flush helpers ----
            def normalize_pre(g, hs):
                # self-loop contribution from the on-chip local table slab
                ls = hs[:, g, :]
                asf = zzp.tile([P, H], BF16, name="asf")
                nc.vector.tensor_add(out=asf[:], in0=ls[:, HID:HID + H],
                                     in1=ls[:, HID + H:HID + 2 * H])
                nc.vector.scalar_tensor_tensor(
                    out=asf[:], in0=asf[:], scalar=0.2, in1=asf[:],
                    op0=mybir.AluOpType.mult, op1=mybir.AluOpType.max)
                es = zzp.tile([P, H], F32, name="es")
                nc.scalar.activation(es[:], asf[:],
                                     mybir.ActivationFunctionType.Exp)
                return ls, es

            def normalize(g, acc, bias_sb, pre):
                ls, es = pre
                # evacuate PSUM acc immediately so the bank frees early
                accs = xbp.tile([P, GC], BF16, name="accs")
                nc.scalar.activation(accs[:], acc[:],
                                     mybir.ActivationFunctionType.Copy)
                den = zzp.tile([P, H], F32, name="den")
                nc.vector.tensor_add(out=den[:], in0=accs[:, HID:GC], in1=es[:])
                rec = zzp.tile([P, H], F32, name="rec")
                nc.vector.reciprocal(rec[:], den[:])
                xb = xbp.tile([P, HID], BF16, name="xb")
                nc.vector.tensor_mul(
                    out=xb[:].rearrange("p (h c) -> p h c", c=cfg.C),
                    in0=ls[:, 0:HID].rearrange("p (h c) -> p h c", c=cfg.C),
                    in1=es[:].to_broadcast([P, H, cfg.C]))
                nc.vector.tensor_add(out=xb[:], in0=xb[:], in1=accs[:, 0:HID])
                nc.vector.tensor_mul(
                    out=xb[:].rearrange("p (h c) -> p h c", c=cfg.C),
                    in0=xb[:].rearrange("p (h c) -> p h c", c=cfg.C),
                    in1=rec[:].to_broadcast([P, H, cfg.C]))
                nc.vector.tensor_add(out=xb[:], in0=xb[:], in1=bias_sb[:])
                nc.vector.tensor_scalar_max(xb[:], xb[:], 0.0)
                return xb

            def transpose2(xb):
                outs = []
                for a in range(HC):
                    pst = ps_tp.tile([P, P], BF16, name="pst")
                    nc.tensor.transpose(pst[:], xb[:, a * P:(a + 1) * P],
                                        identb[:])
                    xts = xtbp.tile([P, P], BF16, name="xts")
                    nc.scalar.activation(xts[:], pst[:],
                                         mybir.ActivationFunctionType.Copy)
                    outs.append(xts)
                return outs

            def flush_layer1(g, acc, pre):
                ng = min(P, npc - g * P)
                xb = normalize(g, acc, b1sb, pre)
                xts = transpose2(xb)
                pstb = ps_tb.tile([P, RT], F32, name="pstb2", tag="pstb")
                for a in range(HC):
                    lh, rh = xts[a][:, 0:ng], M2sb[:, a, :]
                    if f32r:
                        lh, rh = lh.bitcast(F32R), rh.bitcast(F32R)
                    nc.tensor.matmul(pstb[0:ng, :], lh, rh, start=(a == 0),
                                     stop=(a == HC - 1))
                tbs = tblp.tile([P, HID + 2 * H], BF16, name="tbs2")
                if ng < P:
                    nc.vector.memset(tbs[:], 0.0)
                nc.scalar.activation(tbs[0:ng, :], pstb[0:ng, 0:HID + 2 * H],
                                     mybir.ActivationFunctionType.Copy)
                nc.sync.dma_start(
                    out=ag2_in[g * P:(g + 1) * P, 0:HID + 2 * H],
                    in_=tbs[:])
                nc.vector.tensor_copy(out=hs2[:, g, :], in_=tbs[:])

            def flush_layer2(g, acc, pre):
                ng = min(P, npc - g * P)
                xb = normalize(g, acc, b2sb, pre)
                xts = transpose2(xb)
                pslg = ps_tb.tile([P, OUT], F32, name="pslg", tag="pstb")
                for a in range(HC):
                    nc.tensor.matmul(pslg[0:ng, :], xts[a][:, 0:ng],
                                     WcSb[:, a, :], start=(a == 0),
                                     stop=(a == HC - 1))
                lg = hdp.tile([P, OUT], F32, name="lg")
                nc.vector.tensor_add(out=lg[0:ng, :], in0=pslg[0:ng, :],
                                     in1=bcsb[0:ng, :])
                mx = hdp.tile([P, 1], F32, name="mx")
                nc.vector.tensor_reduce(out=mx[0:ng, :], in_=lg[0:ng, :],
                                        axis=mybir.AxisListType.X,
                                        op=mybir.AluOpType.max)
                nc.vector.tensor_sub(out=lg[0:ng, :], in0=lg[0:ng, :],
                                     in1=mx[0:ng, :].to_broadcast([ng, OUT]))
                ex = hdp.tile([P, OUT], F32, name="ex")
                dn = hdp.tile([P, 1], F32, name="dn")
                nc.scalar.activation(ex[0:ng, :], lg[0:ng, :],
                                     mybir.ActivationFunctionType.Exp,
                                     accum_out=dn[0:ng, :])
                lnd = hdp.tile([P, 1], F32, name="lnd")
                nc.scalar.activation(lnd[0:ng, :], dn[0:ng, :],
                                     mybir.ActivationFunctionType.Ln)
                ob = hdp.tile([P, OUT], F32, name="ob")
                nc.vector.tensor_sub(out=ob[0:ng, :], in0=lg[0:ng, :],
                                     in1=lnd[0:ng, :].to_broadcast([ng, OUT]))
                nc.sync.dma_start(out=t_out[g * P:g * P + ng, :],
                                  in_=ob[0:ng, :])

            for _wr in range(whole_reps):
                table1, table2 = tables[_wr]
                phase_b()
                allgather1(table1)
                for _ in range(edge_reps):
                    edge_phase(table1, adst1, flush_layer1, hs1)
                if no_collectives:
                    nc.sync.dma_start(out=table2[0:npc, 0:AGC],
                                      in_=ag2_in[0:npc, 0:AGC])
                else:
                    nc.gpsimd.collective_compute(
                        "AllGather", mybir.AluOpType.bypass,
                        replica_groups=rgroups,
                        ins=[ag2_in[0:npc, :].opt()], outs=[table2.opt()])
                for _ in range(edge_reps):
                    edge_phase(table2, adst2, flush_layer2,
                               hs1 if strip else hs2)

    nc.compile()
    return nc


def make_in_maps(cfg: Cfg, pre, x, W1, as1, ad1, b1, W2, as2, ad2, b2, Wc, bc):
    H, C, HID, npc, RT = cfg.H, cfg.C, cfg.HID, cfg.npc, cfg.RT

    def mk_m(W, a_s, a_d):
        M = np.zeros((W.shape[0], RT), np.float32)
        M[:, 0:HID] = W
        M[:, HID:HID + H] = W @ expand_att(a_s, HID, H, C)
        M[:, HID + H:HID + 2 * H] = W @ expand_att(a_d, HID, H, C)
        return M

    import ml_dtypes
    F8NP = mybir.dt.np(mybir.dt.float8e4)
    BF = ml_dtypes.bfloat16
    M1 = mk_m(W1, as1, ad1).astype(BF)
    M2 = mk_m(W2, as2, ad2).astype(BF)
    maps = []
    for c in range(cfg.n_cores):
        m = {
            "xT": np.ascontiguousarray(x[c * npc:(c + 1) * npc].T).astype(BF),
            "M1": M1, "M2": M2, "Wc": Wc.astype(BF),
            "b1": np.tile(b1[None, :], (P, 1)).astype(np.float32),
            "b2": np.tile(b2[None, :], (P, 1)).astype(np.float32),
            "bc": np.tile(bc[None, :], (P, 1)).astype(np.float32),
            "idx": pre[c]["idx"],
            "sts": pre[c]["sts"].view(F8NP),
        }
        maps.append(m)
    return maps


BUILD_KW = dict(gw_bufs=5, idx_bufs=4, sq=4)


# ---------------------------------------------------------------------------
# Harness entry point: full inputs in, full output out.
# ---------------------------------------------------------------------------

def kernel(x, edge_index, W1, as1, ad1, b1, W2, as2, ad2, b2, Wc, bc):
    x = np.asarray(x, dtype=np.float32)
    edge_index = np.asarray(edge_index)
    N, DIN = x.shape
    H, C = np.asarray(as1).shape
    OUT = np.asarray(Wc).shape[1]
    n_cores = 8

    cfg = Cfg(N, DIN, H, C, OUT, n_cores)
    pre = preprocess(cfg, edge_index)
    nc = build_program(cfg, **BUILD_KW)
    in_maps = make_in_maps(cfg, pre, x,
                           np.asarray(W1, np.float32), np.asarray(as1, np.float32),
                           np.asarray(ad1, np.float32), np.asarray(b1, np.float32),
                           np.asarray(W2, np.float32), np.asarray(as2, np.float32),
                           np.asarray(ad2, np.float32), np.asarray(b2, np.float32),
                           np.asarray(Wc, np.float32), np.asarray(bc, np.float32))

    from concourse import bass_utils
    last_err = None
    for _attempt in range(3):   # a wedged device from a prior crash can fail once
        try:
            res = bass_utils.run_bass_kernel_spmd(nc, in_maps,
                                                  core_ids=list(range(n_cores)))
            break
        except Exception as e:                      # noqa: BLE001
            last_err = e
    else:
        raise last_err
    return np.concatenate([res.results[c]["out"] for c in range(n_cores)],
                          axis=0)

